# revision 4
# baseline (speedup 1.0000x reference)
"""Trainium2 Bass kernel for nn_Decoder: bit-unpack 23x22-bit codes per batch
row, gather fp16 table rows by index, sign-flip about 0.5, scatter into a
[B, 2, 126, 128] fp32 output whose rows 19:67 carry data and the rest are 0.5.

Sharding: data-parallel over batch across 8 NeuronCores (1024 rows each); the
lookup table is replicated on every core.

Table repack (host-side, untimed): the original row is [2, 48, 8] fp16 =
1536B, but codes 0..13 only consume a 4-channel half ([2,48,0:4] for c<7,
[2,48,4:8] for 7<=c<14). We upload TN2[L, 768] fp16 whose row i is
[lo-half(i) | hi-half(i)]; narrow codes gather 768B at element_offset 0/384,
wide codes (14..22) gather the full 1536B row. Cuts gather HBM reads from
35328B to 24576B per batch row with a single 201MB table.

HW indirect gather consumes ONE offset per partition and fetches a contiguous
per-partition block (verified by probe; CoreSim's multi-offset generality does
NOT hold on HW) -> one DMA per code, 23 per group.

Queue layout (a single HWDGE ring tops out ~310GB/s; two rings + SWDGE
sustain 430+): constant fills alternate across BOTH HWDGE rings and nothing
else rides them (x loads go first, before the fills clog the SP ring FIFO).
Gathers ride SWDGE queues 0/1; output stores are fp16->fp32 casting SWDGE
DMAs on queue 2 (cast frees SBUF for a 4-deep od pipeline and takes stores
off the rings entirely). Pool emits store(g-2) after gathers(g) so store
desc-gen never blocks the gather stream and od buffers recycle in time.

Self-contained: hardcodes all shapes; no imports from the problem directory.
"""

import numpy as np

import concourse.bacc as bacc
import concourse.bass as bass
import concourse.mybir as mybir
import concourse.tile as tile

# Problem constants (hardcoded per contract)
BATCH = 8192
XCOLS = 512          # 6 + 23*22
NCODE = 23
NBITS = 22
L = 131072           # table rows
ROW = 768            # fp16 elements per repacked row [lo 384 | hi 384]
HROW = 384
NCORES = 8
BC = BATCH // NCORES  # 1024 rows per core
P = 128
GROUPS = BC // P      # 8 groups of 128 batch rows

# Output geometry: out[b] is [2, 126, 128] fp32 = [p, r, c].
# Data rows are r in [19, 67); flattened per-b layout [32256]:
#   [0:2432) = 0.5 | [2432:8576) p0 data | [8576:18560) = 0.5 |
#   [18560:24704) p1 data | [24704:32256) = 0.5
F_ROW = 126 * 128     # 16128 per p
D_LO = 19 * 128       # 2432
D_HI = 67 * 128       # 8576
GAP_MID = (126 - 67 + 19) * 128   # 9984
GAP_HI = (126 - 67) * 128         # 7552
C05W = GAP_HI // 2    # 3776: fill-source tile width

f16 = mybir.dt.float16
f32 = mybir.dt.float32
i32 = mybir.dt.int32


N_SWDGE_QUEUES = 3
STORE_Q = "qPoolDynamic2"


def build_module():
    nc = bacc.Bacc(
        "TRN2", target_bir_lowering=False, debug=False,
        num_swdge_queues=N_SWDGE_QUEUES,
    )
    x_t = nc.dram_tensor("x", [BC, XCOLS], i32, kind="ExternalInput")
    tn_t = nc.dram_tensor("table", [L, ROW], f16, kind="ExternalInput")
    w_t = nc.dram_tensor("w", [P, NCODE * NBITS], f32, kind="ExternalInput")
    out_t = nc.dram_tensor("out", [BC, 2, 126, 128], f32, kind="ExternalOutput")

    outf = out_t[:].rearrange("b p r c -> b (p r c)")    # [BC, 32256]
    out3 = out_t[:].rearrange("b p r c -> b p (r c)")    # [BC, 2, 16128]

    with tile.TileContext(nc) as tc:
        with (
            tc.tile_pool(name="const", bufs=1) as cpool,
            tc.tile_pool(name="xl", bufs=GROUPS) as xlpool,
            tc.tile_pool(name="xp", bufs=2) as xpool,
            tc.tile_pool(name="sm", bufs=GROUPS) as spool,
            tc.tile_pool(name="gn", bufs=28) as gnpool,
            tc.tile_pool(name="gw", bufs=20) as gwpool,
            tc.tile_pool(name="op", bufs=4) as opool,
        ):
            w_tile = cpool.tile([P, NCODE * NBITS], f32)
            nc.sync.dma_start(w_tile[:], w_t[:])
            c05 = cpool.tile([P, C05W], f32)
            nc.vector.memset(c05[:], 0.5)

            # x loads first: they must hit the SP ring before the fills
            # clog its FIFO (ring drains in issue order).
            x_tiles = []
            for g in range(GROUPS):
                x_tile = xlpool.tile([P, XCOLS], i32)
                nc.sync.dma_start(x_tile[:], x_t[g * P : (g + 1) * P, :])
                x_tiles.append(x_tile)

            # Constant fills: both HWDGE rings, nothing else on them. No
            # data deps -> they stream the whole run keeping HBM write
            # bandwidth busy during gather/compute gaps.
            fill_engs = [nc.sync, nc.scalar]
            nfill = 0

            def fill(dst_lo, dst_hi, g):
                nonlocal nfill
                b0 = g * P
                w = dst_hi - dst_lo
                assert w <= C05W
                fill_engs[nfill % 2].dma_start(
                    out=outf[b0 : b0 + P, dst_lo:dst_hi], in_=c05[:, 0:w]
                )
                nfill += 1

            for g in range(GROUPS):
                fill(0, D_LO, g)
                fill(D_HI, D_HI + C05W, g)
                fill(D_HI + C05W, D_HI + GAP_HI, g)
                fill(D_HI + GAP_HI, D_HI + GAP_MID, g)
                fill(F_ROW + D_HI, F_ROW + D_HI + C05W, g)
                fill(F_ROW + D_HI + C05W, 2 * F_ROW, g)

            # Decode all idx/sign tiles up-front so the gather stream is
            # never gated on the Vector chain mid-flight.
            idxs, tts, sgs = [], [], []
            for g in range(GROUPS):
                xf = xpool.tile([P, XCOLS], f32)
                nc.vector.tensor_copy(out=xf[:], in_=x_tiles[g][:])
                prod = xpool.tile([P, NCODE * NBITS], f32)
                nc.vector.tensor_tensor(
                    out=prod[:], in0=xf[:, 6:], in1=w_tile[:],
                    op=mybir.AluOpType.mult,
                )
                codes = xpool.tile([P, NCODE], f32, tag="codes")
                nc.vector.tensor_reduce(
                    out=codes[:],
                    in_=prod[:].rearrange("n (c a) -> n c a", a=NBITS),
                    axis=mybir.AxisListType.X,
                    op=mybir.AluOpType.add,
                )
                codesi = xpool.tile([P, NCODE], i32, tag="codesi")
                nc.vector.tensor_copy(out=codesi[:], in_=codes[:])
                idx = spool.tile([P, NCODE], i32, tag="idx")
                nc.vector.tensor_scalar(
                    out=idx[:], in0=codesi[:],
                    scalar1=L - 1, scalar2=None,
                    op0=mybir.AluOpType.bitwise_and,
                )
                # tt = 1.0 where codes > L else 0.0 ; sign = 1 - 2*tt
                tt = spool.tile([P, NCODE], f32, tag="tt")
                nc.vector.tensor_scalar(
                    out=tt[:], in0=codes[:],
                    scalar1=float(L), scalar2=None,
                    op0=mybir.AluOpType.is_gt,
                )
                sg = spool.tile([P, NCODE], f32, tag="sg")
                nc.vector.tensor_scalar(
                    out=sg[:], in0=tt[:],
                    scalar1=-2.0, scalar2=1.0,
                    op0=mybir.AluOpType.mult, op1=mybir.AluOpType.add,
                )
                idxs.append(idx); tts.append(tt); sgs.append(sg)

            # Gather + permute + store stream.
            def emit_val(out_ap, in_ap, sg, tt, c):
                # val = sign*g + tt  (== 0.5 + sign*(g-0.5))
                nc.vector.tensor_scalar(
                    out=out_ap, in0=in_ap,
                    scalar1=sg[:, c : c + 1],
                    scalar2=tt[:, c : c + 1],
                    op0=mybir.AluOpType.mult,
                    op1=mybir.AluOpType.add,
                )

            ods = [None] * GROUPS

            def emit_store(g):
                # fp16 -> fp32 cast during DMA: SWDGE only, own queue so
                # stores never contend with gather desc streams.
                si = nc.gpsimd.dma_start(
                    out=out3[g * P : (g + 1) * P, :, D_LO:D_HI],
                    in_=ods[g][:].rearrange("n (p f) -> n p f", p=2),
                )
                si.ins.queue = STORE_Q

            for g in range(GROUPS):
                idx, tt, sg = idxs[g], tts[g], sgs[g]
                od = opool.tile([P, 2 * 48 * 128], f16)
                ods[g] = od
                od4 = od[:].rearrange("n (p k c) -> n p k c", p=2, k=48)
                for c in range(NCODE):
                    wide = c >= 14
                    gc = (gwpool if wide else gnpool).tile(
                        [P, ROW if wide else HROW], f16
                    )
                    gi = nc.gpsimd.indirect_dma_start(
                        out=gc[:],
                        out_offset=None,
                        in_=tn_t[:],
                        in_offset=bass.IndirectOffsetOnAxis(
                            ap=idx[:, c : c + 1], axis=0
                        ),
                        element_offset=HROW if 7 <= c < 14 else 0,
                    )
                    if c % 2:
                        gi.ins.queue = "qPoolDynamic1"
                    if wide:
                        col0 = (c - 7) * 8
                        glo = gc[:, 0:HROW].rearrange(
                            "n (p k c) -> n p k c", p=2, k=48
                        )
                        ghi = gc[:, HROW:ROW].rearrange(
                            "n (p k c) -> n p k c", p=2, k=48
                        )
                        emit_val(od4[:, :, :, col0 : col0 + 4], glo[:], sg, tt, c)
                        emit_val(od4[:, :, :, col0 + 4 : col0 + 8], ghi[:], sg, tt, c)
                    else:
                        col0 = c * 8 if c < 7 else (c - 7) * 8 + 4
                        gv = gc[:].rearrange("n (p k c) -> n p k c", p=2, k=48)
                        emit_val(od4[:, :, :, col0 : col0 + 4], gv[:], sg, tt, c)
                # Lagged store emission on Pool: store desc-gen must not sit
                # ahead of upcoming gather desc-gen in the Pool FIFO.
                if g >= 2:
                    emit_store(g - 2)
            emit_store(GROUPS - 2)
            emit_store(GROUPS - 1)
    nc.compile()
    return nc


def make_weights():
    w = np.tile((2.0 ** np.arange(NBITS)).astype(np.float32), NCODE)
    return np.broadcast_to(w, (P, NCODE * NBITS)).copy()


def make_tn(table):
    t = np.asarray(table).reshape(L, 2, 48, 8)
    tn = np.empty((L, ROW), dtype=np.float16)
    tn[:, :HROW] = t[:, :, :, 0:4].reshape(L, HROW)
    tn[:, HROW:] = t[:, :, :, 4:8].reshape(L, HROW)
    return tn


def make_in_maps(x, table):
    tn = make_tn(table)
    w = make_weights()
    return [
        {
            "x": np.ascontiguousarray(x[i * BC : (i + 1) * BC]),
            "table": tn,
            "w": w,
        }
        for i in range(NCORES)
    ]


_NC_CACHE = None


def _get_module():
    global _NC_CACHE
    if _NC_CACHE is None:
        _NC_CACHE = build_module()
    return _NC_CACHE


def kernel(x: np.ndarray, table: np.ndarray) -> np.ndarray:
    from concourse.bass_utils import run_bass_kernel_spmd

    x = np.asarray(x)
    table = np.asarray(table)
    assert x.shape == (BATCH, XCOLS) and table.shape == (L, 2, 48, 8)
    nc = _get_module()
    res = run_bass_kernel_spmd(nc, make_in_maps(x, table), core_ids=list(range(NCORES)))
    return np.concatenate([res.results[i]["out"] for i in range(NCORES)], axis=0)


# revision 5
# speedup vs baseline: 1.1401x; 1.1401x over previous
"""Trainium2 Bass kernel for nn_Decoder: bit-unpack 23x22-bit codes per batch
row, gather fp16 table rows by index, sign-flip about 0.5, scatter into a
[B, 2, 126, 128] fp32 output whose rows 19:67 carry data and the rest are 0.5.

Sharding: data-parallel over batch across 8 NeuronCores (1024 rows each); the
lookup table is replicated on every core.

Table repack (host-side, untimed): the original row is [2, 48, 8] fp16 =
1536B, but codes 0..13 only consume a 4-channel half ([2,48,0:4] for c<7,
[2,48,4:8] for 7<=c<14). We upload TN2[L, 768] fp16 whose row i is
[lo-half(i) | hi-half(i)]; narrow codes gather 768B at element_offset 0/384,
wide codes (14..22) gather the full 1536B row. Cuts gather HBM reads from
35328B to 24576B per batch row with a single 201MB table.

HW indirect gather consumes ONE offset per partition and fetches a contiguous
per-partition block (verified by probe; CoreSim's multi-offset generality does
NOT hold on HW) -> one DMA per code, 23 per group. Casting SWDGE stores were
tried and are ruinously slow at Q7 desc-gen (~22us per 6MB store) - stores
must stay on HWDGE.

Ring plan (one HWDGE ring tops out ~310GB/s alone; two rings + SWDGE sustain
430+): x loads go first on the SP ring (ring FIFO drains in issue order, so
they must beat the fills in). Each group's output store is split into two
half-stores (p=0 plane on SP, p=1 on ACT) so both rings carry ~67MB and the
od buffer recycles at twice the single-store rate. Constant fills alternate
rings and are interleaved ~3 groups ahead of the half-stores so a store's
semaphore wait never leaves its ring without queued work. Gathers ride SWDGE
queues 0/1; Pool does nothing else.

Self-contained: hardcodes all shapes; no imports from the problem directory.
"""

import numpy as np

import concourse.bacc as bacc
import concourse.bass as bass
import concourse.mybir as mybir
import concourse.tile as tile

# Problem constants (hardcoded per contract)
BATCH = 8192
XCOLS = 512          # 6 + 23*22
NCODE = 23
NBITS = 22
L = 131072           # table rows
ROW = 768            # fp16 elements per repacked row [lo 384 | hi 384]
HROW = 384
NCORES = 8
BC = BATCH // NCORES  # 1024 rows per core
P = 128
GROUPS = BC // P      # 8 groups of 128 batch rows

# Output geometry: out[b] is [2, 126, 128] fp32 = [p, r, c].
# Data rows are r in [19, 67); flattened per-b layout [32256]:
#   [0:2432) = 0.5 | [2432:8576) p0 data | [8576:18560) = 0.5 |
#   [18560:24704) p1 data | [24704:32256) = 0.5
F_ROW = 126 * 128     # 16128 per p
D_LO = 19 * 128       # 2432
D_HI = 67 * 128       # 8576
GAP_MID = (126 - 67 + 19) * 128   # 9984
GAP_HI = (126 - 67) * 128         # 7552
C05W = GAP_HI // 2    # 3776: fill-source tile width
DW = D_HI - D_LO      # 6144: data span per p plane

f16 = mybir.dt.float16
f32 = mybir.dt.float32
i32 = mybir.dt.int32


N_SWDGE_QUEUES = 2


def build_module():
    nc = bacc.Bacc(
        "TRN2", target_bir_lowering=False, debug=False,
        num_swdge_queues=N_SWDGE_QUEUES,
    )
    x_t = nc.dram_tensor("x", [BC, XCOLS], i32, kind="ExternalInput")
    tn_t = nc.dram_tensor("table", [L, ROW], f16, kind="ExternalInput")
    w_t = nc.dram_tensor("w", [P, NCODE * NBITS], f32, kind="ExternalInput")
    out_t = nc.dram_tensor("out", [BC, 2, 126, 128], f32, kind="ExternalOutput")

    outf = out_t[:].rearrange("b p r c -> b (p r c)")    # [BC, 32256]

    with tile.TileContext(nc) as tc:
        with (
            tc.tile_pool(name="const", bufs=1) as cpool,
            tc.tile_pool(name="xl", bufs=GROUPS) as xlpool,
            tc.tile_pool(name="xp", bufs=2) as xpool,
            tc.tile_pool(name="sm", bufs=GROUPS) as spool,
            tc.tile_pool(name="gn", bufs=28) as gnpool,
            tc.tile_pool(name="gw", bufs=20) as gwpool,
            tc.tile_pool(name="op", bufs=2) as opool,
        ):
            w_tile = cpool.tile([P, NCODE * NBITS], f32)
            nc.sync.dma_start(w_tile[:], w_t[:])
            c05 = cpool.tile([P, C05W], f32)
            nc.vector.memset(c05[:], 0.5)

            # x loads first: they must hit the SP ring before the fills
            # clog its FIFO (ring drains in issue order).
            x_tiles = []
            for g in range(GROUPS):
                x_tile = xlpool.tile([P, XCOLS], i32)
                nc.sync.dma_start(x_tile[:], x_t[g * P : (g + 1) * P, :])
                x_tiles.append(x_tile)

            # Constant fills, alternating rings. Emitted via fills_for(g)
            # interleaved ahead of the half-stores in the main loop.
            fill_engs = [nc.sync, nc.scalar]
            nfill = 0

            def fill(dst_lo, dst_hi, g):
                nonlocal nfill
                b0 = g * P
                w = dst_hi - dst_lo
                assert w <= C05W
                fill_engs[nfill % 2].dma_start(
                    out=outf[b0 : b0 + P, dst_lo:dst_hi], in_=c05[:, 0:w]
                )
                nfill += 1

            def fills_for(g):
                fill(0, D_LO, g)
                fill(D_HI, D_HI + C05W, g)
                fill(D_HI + C05W, D_HI + GAP_HI, g)
                fill(D_HI + GAP_HI, D_HI + GAP_MID, g)
                fill(F_ROW + D_HI, F_ROW + D_HI + C05W, g)
                fill(F_ROW + D_HI + C05W, 2 * F_ROW, g)

            # Decode all idx/sign tiles up-front so the gather stream is
            # never gated on the Vector chain mid-flight.
            idxs, tts, sgs = [], [], []
            for g in range(GROUPS):
                xf = xpool.tile([P, XCOLS], f32)
                nc.vector.tensor_copy(out=xf[:], in_=x_tiles[g][:])
                prod = xpool.tile([P, NCODE * NBITS], f32)
                nc.vector.tensor_tensor(
                    out=prod[:], in0=xf[:, 6:], in1=w_tile[:],
                    op=mybir.AluOpType.mult,
                )
                codes = xpool.tile([P, NCODE], f32, tag="codes")
                nc.vector.tensor_reduce(
                    out=codes[:],
                    in_=prod[:].rearrange("n (c a) -> n c a", a=NBITS),
                    axis=mybir.AxisListType.X,
                    op=mybir.AluOpType.add,
                )
                codesi = xpool.tile([P, NCODE], i32, tag="codesi")
                nc.vector.tensor_copy(out=codesi[:], in_=codes[:])
                idx = spool.tile([P, NCODE], i32, tag="idx")
                nc.vector.tensor_scalar(
                    out=idx[:], in0=codesi[:],
                    scalar1=L - 1, scalar2=None,
                    op0=mybir.AluOpType.bitwise_and,
                )
                # tt = 1.0 where codes > L else 0.0 ; sign = 1 - 2*tt
                tt = spool.tile([P, NCODE], f32, tag="tt")
                nc.vector.tensor_scalar(
                    out=tt[:], in0=codes[:],
                    scalar1=float(L), scalar2=None,
                    op0=mybir.AluOpType.is_gt,
                )
                sg = spool.tile([P, NCODE], f32, tag="sg")
                nc.vector.tensor_scalar(
                    out=sg[:], in0=tt[:],
                    scalar1=-2.0, scalar2=1.0,
                    op0=mybir.AluOpType.mult, op1=mybir.AluOpType.add,
                )
                idxs.append(idx); tts.append(tt); sgs.append(sg)

            # Prime both rings with 3 groups of fills before any store can
            # park a semaphore wait in front of them.
            for g in range(3):
                fills_for(g)

            # Gather + permute + store stream.
            def emit_val(out_ap, in_ap, sg, tt, c):
                # val = sign*g + tt  (== 0.5 + sign*(g-0.5))
                nc.vector.tensor_scalar(
                    out=out_ap, in0=in_ap,
                    scalar1=sg[:, c : c + 1],
                    scalar2=tt[:, c : c + 1],
                    op0=mybir.AluOpType.mult,
                    op1=mybir.AluOpType.add,
                )

            for g in range(GROUPS):
                b0 = g * P
                idx, tt, sg = idxs[g], tts[g], sgs[g]
                od = opool.tile([P, 2 * DW], f32)
                od4 = od[:].rearrange("n (p k c) -> n p k c", p=2, k=48)
                for c in range(NCODE):
                    wide = c >= 14
                    gc = (gwpool if wide else gnpool).tile(
                        [P, ROW if wide else HROW], f16
                    )
                    gi = nc.gpsimd.indirect_dma_start(
                        out=gc[:],
                        out_offset=None,
                        in_=tn_t[:],
                        in_offset=bass.IndirectOffsetOnAxis(
                            ap=idx[:, c : c + 1], axis=0
                        ),
                        element_offset=HROW if 7 <= c < 14 else 0,
                    )
                    if c % 2:
                        gi.ins.queue = "qPoolDynamic1"
                    if wide:
                        col0 = (c - 7) * 8
                        glo = gc[:, 0:HROW].rearrange(
                            "n (p k c) -> n p k c", p=2, k=48
                        )
                        ghi = gc[:, HROW:ROW].rearrange(
                            "n (p k c) -> n p k c", p=2, k=48
                        )
                        emit_val(od4[:, :, :, col0 : col0 + 4], glo[:], sg, tt, c)
                        emit_val(od4[:, :, :, col0 + 4 : col0 + 8], ghi[:], sg, tt, c)
                    else:
                        col0 = c * 8 if c < 7 else (c - 7) * 8 + 4
                        gv = gc[:].rearrange("n (p k c) -> n p k c", p=2, k=48)
                        emit_val(od4[:, :, :, col0 : col0 + 4], gv[:], sg, tt, c)
                if g + 3 < GROUPS:
                    fills_for(g + 3)
                # Half-stores: p0 plane on SP ring, p1 plane on ACT ring.
                nc.sync.dma_start(
                    out=outf[b0 : b0 + P, D_LO:D_HI], in_=od[:, 0:DW]
                )
                nc.scalar.dma_start(
                    out=outf[b0 : b0 + P, F_ROW + D_LO : F_ROW + D_HI],
                    in_=od[:, DW : 2 * DW],
                )
    nc.compile()
    return nc


def make_weights():
    w = np.tile((2.0 ** np.arange(NBITS)).astype(np.float32), NCODE)
    return np.broadcast_to(w, (P, NCODE * NBITS)).copy()


def make_tn(table):
    t = np.asarray(table).reshape(L, 2, 48, 8)
    tn = np.empty((L, ROW), dtype=np.float16)
    tn[:, :HROW] = t[:, :, :, 0:4].reshape(L, HROW)
    tn[:, HROW:] = t[:, :, :, 4:8].reshape(L, HROW)
    return tn


def make_in_maps(x, table):
    tn = make_tn(table)
    w = make_weights()
    return [
        {
            "x": np.ascontiguousarray(x[i * BC : (i + 1) * BC]),
            "table": tn,
            "w": w,
        }
        for i in range(NCORES)
    ]


_NC_CACHE = None


def _get_module():
    global _NC_CACHE
    if _NC_CACHE is None:
        _NC_CACHE = build_module()
    return _NC_CACHE


def kernel(x: np.ndarray, table: np.ndarray) -> np.ndarray:
    from concourse.bass_utils import run_bass_kernel_spmd

    x = np.asarray(x)
    table = np.asarray(table)
    assert x.shape == (BATCH, XCOLS) and table.shape == (L, 2, 48, 8)
    nc = _get_module()
    res = run_bass_kernel_spmd(nc, make_in_maps(x, table), core_ids=list(range(NCORES)))
    return np.concatenate([res.results[i]["out"] for i in range(NCORES)], axis=0)
